# revision 2
# baseline (speedup 1.0000x reference)
"""Trainium2 Bass kernel for nn_ConvCNF: D4-symmetric periodic conv CNF layer.

Math (matching the reference):
  feats_f(x) = sin(f+1 * x) for f<49, feats_49 = x          (per pixel)
  ck[o, f]  = sum_i kp[o, i] * fs[i, f]                      (3, 50)
  grad[b,h,w] = sum_over_3x3_stencil ck[orbit(dy,dx), f] * feats_f(neighbor)
  -div[b]   = sum_pixels sum_k (-k * w00[k-1]) * cos(k x) - w00[49]*H*W

Device decomposition (per pixel tile, partition dim = h):
  P = (ck0*I + ck1*N) @ feats   (N = circulant h-neighbor matrix, wrap incl.)
  Q = (ck1*I + ck2*N) @ feats
  grad = P + Q(w-1) + Q(w+1)    (w-shifts on DVE, periodic)
Sharding: batch across 8 cores (16 images each). Parameters replicated.

sin(kx) needs explicit range reduction (ACT Sin domain is [-pi, pi]):
  r = (k*y + phase) - rint(...) via a custom fused DVE op (magic-add rint),
  y = x/(2pi), phase = 0 for sin / 0.25 for cos; then ACT sin(2pi * r).
"""

import sys

import numpy as np

sys.path.insert(0, "/opt/trn_rl_repo")

from concourse import bacc, mybir  # noqa: E402
from concourse.bass_utils import run_bass_kernel_spmd  # noqa: E402
from concourse.tile import TileContext  # noqa: E402

# ---------------------------------------------------------------- custom DVE op
from concourse.dve_ops import (  # noqa: E402
    CUSTOM_DVE_SPECS,
    OPS,
    DveOp,
    _CUSTOM_DVE_ROW_BASE,
    _SUB_OPCODE_FOR_NAME,
)
from concourse.dve_spec import C0, C1, C2, Spec, Src0, lower  # noqa: E402
from concourse.dve_uop import DveOpSpec  # noqa: E402

MAGIC = float(1.5 * 2**23)  # fl(p + M) - M == rint(p) for |p| < 2^21
_FRAC_NAME = "FRAC_RED_ANT"


def _frac_ref(in0, s0, s1, imm2):
    p = (np.asarray(in0, np.float32) * np.float32(s0) + np.float32(s1)).astype(
        np.float32
    )
    n = ((p + np.float32(imm2)) - np.float32(imm2)).astype(np.float32)
    return (p - n).astype(np.float32)


def _register_frac_red() -> DveOp:
    for op in OPS:
        if op.name == _FRAC_NAME:
            return op
    p = Src0 * C0 + C1
    spec = Spec(body=p - ((p + C2) - C2), reference=_frac_ref)
    opcode = _CUSTOM_DVE_ROW_BASE + len(OPS)
    assert opcode < 0x20
    _SUB_OPCODE_FOR_NAME[_FRAC_NAME] = opcode
    uops = lower(spec, ver="v3")
    sha = DveOpSpec(name=_FRAC_NAME, opcode=opcode, uops=uops, rd1_en=False).sha("v3")
    op = DveOp(_FRAC_NAME, spec, subdim=False, uops_sha={"v3": sha})
    OPS.append(op)
    CUSTOM_DVE_SPECS[_FRAC_NAME] = spec
    return op


FRAC_RED = _register_frac_red()

# ------------------------------------------------------------------- constants
B, H, W = 128, 128, 128
NCORES = 8
BPC = B // NCORES  # images per core
NK = 49  # sin harmonics
NF = NK + 1  # features incl. the linear x feature
TWO_PI = float(2 * np.pi)
# sin(2pi * r) with r in [-0.5, 0.5]; shrink so |arg| stays strictly < fp32 pi
SIN_SCALE = float(np.float32(TWO_PI) * (1.0 - 2.0**-22))

F32 = mybir.dt.float32
F32R = mybir.dt.float32r
SIN = mybir.ActivationFunctionType.Sin

_PROGRAM_CACHE = {}


def _build_program():
    if "nc" in _PROGRAM_CACHE:
        return _PROGRAM_CACHE["nc"]

    nc = bacc.Bacc("TRN2", target_bir_lowering=False)

    y1_t = nc.dram_tensor("y1", [BPC, H, W], F32, kind="ExternalInput")
    wp_t = nc.dram_tensor("wp", [NF, H, H], F32R, kind="ExternalInput")
    wq_t = nc.dram_tensor("wq", [NF, H, H], F32R, kind="ExternalInput")
    wd_t = nc.dram_tensor("wd", [H, NK], F32R, kind="ExternalInput")
    grad_t = nc.dram_tensor("grad_out", [BPC, H, W], F32, kind="ExternalOutput")
    div_t = nc.dram_tensor("div_out", [1, BPC], F32, kind="ExternalOutput")

    with TileContext(nc) as tc:
        with tc.tile_pool(name="persist", bufs=1) as keep:
            ytile = keep.tile([H, BPC, W], F32)
            nc.sync.dma_start(ytile[:], y1_t[:].rearrange("b h w -> h b w"))
            wdt = keep.tile([H, NK], F32R)
            nc.sync.dma_start(wdt[:], wd_t[:])
            gtile = keep.tile([H, BPC, W], F32)
            dcol = keep.tile([1, BPC], F32)

            # ---------------- phase A: grad accumulation over 50 features
            with tc.tile_pool(name="psA", bufs=1, space="PSUM") as psA, \
                 tc.tile_pool(name="wts", bufs=4) as wpool, \
                 tc.tile_pool(name="red", bufs=3) as rpool, \
                 tc.tile_pool(name="sines", bufs=3) as spool:
                pp = psA.tile([H, BPC, W], F32)
                pq = psA.tile([H, BPC, W], F32)
                for j in range(NF):
                    s_j = spool.tile([H, BPC, W], F32R, tag="s")
                    if j == 0:
                        # linear feature: s = x = 2pi * y
                        nc.vector.tensor_scalar_mul(s_j[:], ytile[:], TWO_PI)
                    else:
                        r_j = rpool.tile([H, BPC, W], F32, tag="r")
                        nc.vector._custom_dve(
                            FRAC_RED, out=r_j[:], in0=ytile[:],
                            s0=float(j), s1=0.0, imm2=MAGIC,
                        )
                        nc.scalar.activation(s_j[:], r_j[:], SIN, scale=SIN_SCALE)
                    wpt = wpool.tile([H, H], F32R, tag="wp")
                    wqt = wpool.tile([H, H], F32R, tag="wq")
                    nc.sync.dma_start(wpt[:], wp_t[j])
                    nc.sync.dma_start(wqt[:], wq_t[j])
                    first, last = j == 0, j == NF - 1
                    for h in range(0, BPC, 4):  # 512-col chunks (1 PSUM bank)
                        nc.tensor.matmul(
                            pp[:, h : h + 4, :], wpt[:], s_j[:, h : h + 4, :],
                            start=first, stop=last,
                        )
                        nc.tensor.matmul(
                            pq[:, h : h + 4, :], wqt[:], s_j[:, h : h + 4, :],
                            start=first, stop=last,
                        )

                # stencil: grad = P + Q(w-1) + Q(w+1), periodic in w.
                # DVE has a single PSUM read port, so P is copied to SBUF
                # first and the Q shifts accumulate in place (1 PSUM src/op).
                add = mybir.AluOpType.add
                nc.vector.tensor_copy(gtile[:], pp[:])
                nc.vector.tensor_tensor(
                    gtile[:, :, 1:W], gtile[:, :, 1:W], pq[:, :, 0 : W - 1], add
                )
                nc.vector.tensor_tensor(
                    gtile[:, :, 0:1], gtile[:, :, 0:1], pq[:, :, W - 1 : W], add
                )
                nc.vector.tensor_tensor(
                    gtile[:, :, 0 : W - 1], gtile[:, :, 0 : W - 1], pq[:, :, 1:W], add
                )
                nc.vector.tensor_tensor(
                    gtile[:, :, W - 1 : W], gtile[:, :, W - 1 : W], pq[:, :, 0:1], add
                )
            nc.sync.dma_start(grad_t[:].rearrange("b h w -> h b w"), gtile[:])

            # ---------------- phase B: divergence (cos features)
            with tc.tile_pool(name="psB", bufs=1, space="PSUM") as psB, \
                 tc.tile_pool(name="redB", bufs=3) as rpoolB, \
                 tc.tile_pool(name="sinesB", bufs=3) as spoolB:
                pd = psB.tile([1, BPC, W], F32)
                for j in range(1, NK + 1):
                    rc_j = rpoolB.tile([H, BPC, W], F32, tag="rc")
                    nc.vector._custom_dve(
                        FRAC_RED, out=rc_j[:], in0=ytile[:],
                        s0=float(j), s1=0.25, imm2=MAGIC,
                    )
                    c_j = spoolB.tile([H, BPC, W], F32R, tag="c")
                    nc.scalar.activation(c_j[:], rc_j[:], SIN, scale=SIN_SCALE)
                    first, last = j == 1, j == NK
                    for h in range(0, BPC, 4):
                        nc.tensor.matmul(
                            pd[:, h : h + 4, :], wdt[:, j - 1 : j],
                            c_j[:, h : h + 4, :], start=first, stop=last,
                        )
                nc.vector.tensor_reduce(
                    dcol[:], pd[:], mybir.AxisListType.X, mybir.AluOpType.add
                )
            nc.sync.dma_start(div_t[:], dcol[:])

    nc.compile()
    _PROGRAM_CACHE["nc"] = nc
    return nc


def kernel(t, x, time_red, kernel_params, feature_superposition):
    t = np.float64(np.asarray(t))
    x = np.asarray(x, dtype=np.float32)
    time_red = np.asarray(time_red, dtype=np.float64)
    kernel_params = np.asarray(kernel_params, dtype=np.float64)
    feature_superposition = np.asarray(feature_superposition, dtype=np.float64)

    # ---- tiny host-side parameter math (fp64) ----
    k10 = np.arange(1, 11, dtype=np.float64)
    base = np.concatenate(
        [[1.0], np.cos(2 * np.pi * k10 * t), np.sin(2 * np.pi * k10 * t)]
    )  # (21,)
    t_emb = time_red @ base  # (20,)
    kp = np.einsum(
        "oict,t->oic", kernel_params.reshape(3, 20, 1, 20), t_emb
    )[:, :, 0]  # (3, 20)
    fs = feature_superposition / 50.0  # (20, 50)
    w00 = fs.T @ kp[0]  # (50,)
    ck = kp @ fs  # (3, 50): combined weights per orbit x feature

    # circulant h-neighbor matrix (includes periodic wrap)
    eye = np.eye(H, dtype=np.float64)
    nbr = np.roll(eye, 1, axis=1) + np.roll(eye, -1, axis=1)

    # feature order on device: j=0 -> linear x feature (index 49), j>=1 -> sin(j x)
    fidx = np.concatenate([[NF - 1], np.arange(NK)])  # (50,)
    a = ck[0, fidx]  # center weights
    b = ck[1, fidx]  # edge weights
    c = ck[2, fidx]  # corner weights
    wp = (
        a[:, None, None] * eye[None] + b[:, None, None] * nbr[None]
    ).astype(np.float32)  # (50, 128, 128)
    wq = (
        b[:, None, None] * eye[None] + c[:, None, None] * nbr[None]
    ).astype(np.float32)
    kvec = np.arange(1, NK + 1, dtype=np.float64)
    wd = np.broadcast_to(
        (-kvec * w00[:NK]).astype(np.float32), (H, NK)
    ).copy()  # (128, 49)

    y1 = (x.astype(np.float64) / (2 * np.pi)).astype(np.float32)

    nc = _build_program()
    in_maps = [
        {
            "y1": np.ascontiguousarray(y1[core * BPC : (core + 1) * BPC]),
            "wp": wp,
            "wq": wq,
            "wd": wd,
        }
        for core in range(NCORES)
    ]
    out = run_bass_kernel_spmd(nc, in_maps, core_ids=list(range(NCORES)))

    grad = np.concatenate(
        [out.results[core]["grad_out"] for core in range(NCORES)], axis=0
    ).astype(np.float32)
    dev_div = np.concatenate(
        [out.results[core]["div_out"][0] for core in range(NCORES)], axis=0
    ).astype(np.float64)
    neg_div = (dev_div - w00[NF - 1] * H * W).astype(np.float32)
    return grad, neg_div


# revision 3
# speedup vs baseline: 1.3214x; 1.3214x over previous
"""Trainium2 Bass kernel for nn_ConvCNF: D4-symmetric periodic conv CNF layer.

Device decomposition (per pixel tile, partition dim = h):
  P = (ck0*I + ck1*N) @ feats   (N = circulant h-neighbor matrix, wrap incl.)
  Q = (ck1*I + ck2*N) @ feats
  grad = P + Q(w-1) + Q(w+1)    (w-shifts on DVE, periodic)
Divergence: cos features evaluated on a second copy of y laid out with
partition=(batch, h-block); ACT accum_out gives per-partition sums, the
tiny (128, 49) result is reduced+weighted on the host.
Sharding: batch across 8 cores (16 images each). Parameters replicated.

sin(kx) range reduction (ACT Sin domain is [-pi, pi]):
  r = (k*y + phase) - rint(...) via a custom fused DVE op (magic-add rint),
  y = x/(2pi), phase = 0 for sin / 0.25 for cos; then ACT sin(2pi * r).
  A slice of the cos reductions runs on GpSimd (stock 3-op chain) to
  balance engine load.
"""

import sys

import numpy as np

sys.path.insert(0, "/opt/trn_rl_repo")

from concourse import bacc, mybir  # noqa: E402
from concourse.bass_utils import run_bass_kernel_spmd  # noqa: E402
from concourse.tile import TileContext  # noqa: E402

# ---------------------------------------------------------------- custom DVE op
from concourse.dve_ops import (  # noqa: E402
    CUSTOM_DVE_SPECS,
    OPS,
    DveOp,
    _CUSTOM_DVE_ROW_BASE,
    _SUB_OPCODE_FOR_NAME,
)
from concourse.dve_spec import C0, C1, C2, Spec, Src0, lower  # noqa: E402
from concourse.dve_uop import DveOpSpec  # noqa: E402

MAGIC = float(1.5 * 2**23)  # fl(p + M) - M == rint(p) for |p| < 2^21
_FRAC_NAME = "FRAC_RED_ANT"


def _frac_ref(in0, s0, s1, imm2):
    p = (np.asarray(in0, np.float32) * np.float32(s0) + np.float32(s1)).astype(
        np.float32
    )
    n = ((p + np.float32(imm2)) - np.float32(imm2)).astype(np.float32)
    return (p - n).astype(np.float32)


def _register_frac_red() -> DveOp:
    for op in OPS:
        if op.name == _FRAC_NAME:
            return op
    p = Src0 * C0 + C1
    spec = Spec(body=p - ((p + C2) - C2), reference=_frac_ref)
    opcode = _CUSTOM_DVE_ROW_BASE + len(OPS)
    assert opcode < 0x20
    _SUB_OPCODE_FOR_NAME[_FRAC_NAME] = opcode
    uops = lower(spec, ver="v3")
    sha = DveOpSpec(name=_FRAC_NAME, opcode=opcode, uops=uops, rd1_en=False).sha("v3")
    op = DveOp(_FRAC_NAME, spec, subdim=False, uops_sha={"v3": sha})
    OPS.append(op)
    CUSTOM_DVE_SPECS[_FRAC_NAME] = spec
    return op


FRAC_RED = _register_frac_red()

# ------------------------------------------------------------------- constants
B, H, W = 128, 128, 128
NCORES = 8
BPC = B // NCORES  # images per core
NK = 49  # sin harmonics
NF = NK + 1  # features incl. the linear x feature
TWO_PI = float(2 * np.pi)
SIN_SCALE = float(np.float32(TWO_PI) * (1.0 - 2.0**-22))
# how many of the 49 cos-reduction chains run on GpSimd instead of DVE
GPSIMD_COS = 26

F32 = mybir.dt.float32
F32R = mybir.dt.float32r
SIN = mybir.ActivationFunctionType.Sin
ADD = mybir.AluOpType.add
SUB = mybir.AluOpType.subtract
MUL = mybir.AluOpType.mult

_PROGRAM_CACHE = {}


def _build_program():
    if "nc" in _PROGRAM_CACHE:
        return _PROGRAM_CACHE["nc"]

    nc = bacc.Bacc("TRN2", target_bir_lowering=False)

    y1_t = nc.dram_tensor("y1", [BPC, H, W], F32, kind="ExternalInput")
    wp_t = nc.dram_tensor("wp", [NF, H, H], F32R, kind="ExternalInput")
    wq_t = nc.dram_tensor("wq", [NF, H, H], F32R, kind="ExternalInput")
    grad_t = nc.dram_tensor("grad_out", [BPC, H, W], F32, kind="ExternalOutput")
    diva_t = nc.dram_tensor("diva_out", [H, NK], F32, kind="ExternalOutput")

    with TileContext(nc) as tc:
        with tc.tile_pool(name="persist", bufs=1) as keep:
            ytile = keep.tile([H, BPC, W], F32)
            nc.sync.dma_start(ytile[:], y1_t[:].rearrange("b h w -> h b w"))
            # second copy, partition = (b, h/16): row p=b*8+hh holds rows
            # 16*hh..16*hh+15 of image b  -> accum_out sums stay per-batch
            ytile2 = keep.tile([H, BPC, W], F32)
            for b in range(BPC):
                nc.sync.dma_start(
                    ytile2[8 * b : 8 * b + 8],
                    y1_t[b].rearrange("(hh hl) w -> hh (hl w)", hh=8)
                    .rearrange("hh (hl w) -> hh hl w", hl=BPC),
                )
            gtile = keep.tile([H, BPC, W], F32)
            atile = keep.tile([H, NK], F32)

            with tc.tile_pool(name="ps", bufs=1, space="PSUM") as psA, \
                 tc.tile_pool(name="wts", bufs=4) as wpool, \
                 tc.tile_pool(name="red", bufs=3) as rpool, \
                 tc.tile_pool(name="sines", bufs=3) as spool, \
                 tc.tile_pool(name="gs", bufs=3) as gpool:
                pp = psA.tile([H, BPC, W], F32)
                pq = psA.tile([H, BPC, W], F32)
                for j in range(NF):
                    # ---- sin feature tile (partition = h) for the conv
                    s_j = spool.tile([H, BPC, W], F32R, tag="s")
                    if j == 0:
                        nc.scalar.mul(s_j[:], ytile[:], TWO_PI)  # linear feature
                    else:
                        r_j = rpool.tile([H, BPC, W], F32, tag="r")
                        nc.vector._custom_dve(
                            FRAC_RED, out=r_j[:], in0=ytile[:],
                            s0=float(j), s1=0.0, imm2=MAGIC,
                        )
                        nc.scalar.activation(s_j[:], r_j[:], SIN, scale=SIN_SCALE)
                    wpt = wpool.tile([H, H], F32R, tag="wp")
                    wqt = wpool.tile([H, H], F32R, tag="wq")
                    nc.sync.dma_start(wpt[:], wp_t[j])
                    nc.sync.dma_start(wqt[:], wq_t[j])
                    first, last = j == 0, j == NF - 1
                    for h in range(0, BPC, 4):  # 512-col chunks (1 PSUM bank)
                        nc.tensor.matmul(
                            pp[:, h : h + 4, :], wpt[:], s_j[:, h : h + 4, :],
                            start=first, stop=last,
                        )
                        nc.tensor.matmul(
                            pq[:, h : h + 4, :], wqt[:], s_j[:, h : h + 4, :],
                            start=first, stop=last,
                        )

                    # ---- cos feature on ytile2, reduced via ACT accum_out
                    if j >= 1:
                        rc_j = rpool.tile([H, BPC, W], F32, tag="rc")
                        if j <= GPSIMD_COS:
                            # stock 3-op chain on GpSimd to offload DVE
                            pg = gpool.tile([H, BPC, W], F32, tag="pg")
                            ng = gpool.tile([H, BPC, W], F32, tag="ng")
                            nc.gpsimd.tensor_scalar(
                                pg[:], ytile2[:], float(j), 0.25, MUL, ADD
                            )
                            nc.gpsimd.tensor_scalar(
                                ng[:], pg[:], MAGIC, MAGIC, ADD, SUB
                            )
                            nc.gpsimd.tensor_tensor(rc_j[:], pg[:], ng[:], SUB)
                        else:
                            nc.vector._custom_dve(
                                FRAC_RED, out=rc_j[:], in0=ytile2[:],
                                s0=float(j), s1=0.25, imm2=MAGIC,
                            )
                        cscr = spool.tile([H, BPC, W], F32, tag="c")
                        nc.scalar.activation(
                            cscr[:], rc_j[:], SIN, scale=SIN_SCALE,
                            accum_out=atile[:, j - 1 : j],
                        )

                # stencil: grad = P + Q(w-1) + Q(w+1), periodic in w.
                # P copy on ACT (has slack); Q shifts accumulate in place on
                # DVE (single PSUM source per op).
                nc.scalar.copy(gtile[:], pp[:])
                nc.vector.tensor_tensor(
                    gtile[:, :, 1:W], gtile[:, :, 1:W], pq[:, :, 0 : W - 1], ADD
                )
                nc.vector.tensor_tensor(
                    gtile[:, :, 0:1], gtile[:, :, 0:1], pq[:, :, W - 1 : W], ADD
                )
                nc.vector.tensor_tensor(
                    gtile[:, :, 0 : W - 1], gtile[:, :, 0 : W - 1], pq[:, :, 1:W], ADD
                )
                nc.vector.tensor_tensor(
                    gtile[:, :, W - 1 : W], gtile[:, :, W - 1 : W], pq[:, :, 0:1], ADD
                )
            nc.sync.dma_start(grad_t[:].rearrange("b h w -> h b w"), gtile[:])
            nc.sync.dma_start(diva_t[:], atile[:])

    nc.compile()
    _PROGRAM_CACHE["nc"] = nc
    return nc


def kernel(t, x, time_red, kernel_params, feature_superposition):
    t = np.float64(np.asarray(t))
    x = np.asarray(x, dtype=np.float32)
    time_red = np.asarray(time_red, dtype=np.float64)
    kernel_params = np.asarray(kernel_params, dtype=np.float64)
    feature_superposition = np.asarray(feature_superposition, dtype=np.float64)

    # ---- tiny host-side parameter math (fp64) ----
    k10 = np.arange(1, 11, dtype=np.float64)
    base = np.concatenate(
        [[1.0], np.cos(2 * np.pi * k10 * t), np.sin(2 * np.pi * k10 * t)]
    )  # (21,)
    t_emb = time_red @ base  # (20,)
    kp = np.einsum(
        "oict,t->oic", kernel_params.reshape(3, 20, 1, 20), t_emb
    )[:, :, 0]  # (3, 20)
    fs = feature_superposition / 50.0  # (20, 50)
    w00 = fs.T @ kp[0]  # (50,)
    ck = kp @ fs  # (3, 50): combined weights per orbit x feature

    eye = np.eye(H, dtype=np.float64)
    nbr = np.roll(eye, 1, axis=1) + np.roll(eye, -1, axis=1)

    # feature order on device: j=0 -> linear x feature (index 49), j>=1 -> sin(j x)
    fidx = np.concatenate([[NF - 1], np.arange(NK)])  # (50,)
    a = ck[0, fidx]
    b = ck[1, fidx]
    c = ck[2, fidx]
    wp = (a[:, None, None] * eye[None] + b[:, None, None] * nbr[None]).astype(
        np.float32
    )
    wq = (b[:, None, None] * eye[None] + c[:, None, None] * nbr[None]).astype(
        np.float32
    )

    y1 = (x.astype(np.float64) / (2 * np.pi)).astype(np.float32)

    nc = _build_program()
    in_maps = [
        {
            "y1": np.ascontiguousarray(y1[core * BPC : (core + 1) * BPC]),
            "wp": wp,
            "wq": wq,
        }
        for core in range(NCORES)
    ]
    out = run_bass_kernel_spmd(nc, in_maps, core_ids=list(range(NCORES)))

    grad = np.concatenate(
        [out.results[core]["grad_out"] for core in range(NCORES)], axis=0
    ).astype(np.float32)

    # diva rows p = b_local*8 + hh hold per-(batch, h-block) cos sums
    kvec = np.arange(1, NK + 1, dtype=np.float64)
    kw = kvec * w00[:NK]  # (49,)
    neg_div = np.empty(B, dtype=np.float64)
    for core in range(NCORES):
        diva = out.results[core]["diva_out"].astype(np.float64)  # (128, 49)
        per_batch = diva.reshape(BPC, 8, NK).sum(axis=1)  # (16, 49)
        neg_div[core * BPC : (core + 1) * BPC] = -per_batch @ kw - w00[NF - 1] * H * W
    return grad, neg_div.astype(np.float32)
